# revision 27
# baseline (speedup 1.0000x reference)
"""Distributed causal multi-head attention for 8 TRN2 NeuronCores.

Sharding: 24 (batch, head) units -> 8 cores x 3 heads (tensor parallel over
heads, data parallel over batch; cores 0-3 = batch 0, cores 4-7 = batch 1).
Per core: project q/k/v for its 3 heads (f32r matmuls), causal softmax without
max-subtraction (scores are bounded), attention, per-head normalization, then
an 8-core AllGather of the per-head outputs zT and a query-sharded output
projection (each core emits 512 rows of attn_out for its batch).
"""
import sys
import math
import numpy as np

sys.path.insert(0, "/opt/trn_rl_repo")

D_MODEL, N_HEADS, D_HEAD = 768, 12, 64
BATCH, SEQ = 2, 2048
HPC = 3              # heads per core
GROUP = 4            # cores per batch group
N_CORES = 8
CHQ = 512            # query-chunk width (softmax/attention N dim)
KT = 128             # key-tile height
NQC = SEQ // CHQ     # 4
NKQ = CHQ // KT      # boundary k-tiles per q-chunk: 4
NT = SEQ // KT       # 16 token tiles
ND = D_MODEL // 128  # 6 contraction chunks
SCALE = 1.0 / math.sqrt(D_HEAD)

# qkv packing into 5 [128, SEQ] tiles; slot s -> tile s//2, base 64*(s%2).
# Order chosen so q_h and k_h always share the same base partition.
SLOTS = [("q", 0), ("q", 1), ("k", 0), ("k", 1), ("q", 2),
         ("v", 0), ("k", 2), ("v", 1), ("v", 2)]
SLOT = {key: s for s, key in enumerate(SLOTS)}

_BUILT = None


def _build():
    import concourse.bass as bass
    import concourse.bacc as bacc
    import concourse.mybir as mybir
    import concourse.tile as tile
    from concourse.masks import make_identity

    f32, f32r = mybir.dt.float32, mybir.dt.float32r
    bf16 = mybir.dt.bfloat16
    FT = mybir.ActivationFunctionType

    nc = bacc.Bacc("TRN2", target_bir_lowering=False, debug=False,
                   num_devices=N_CORES)

    x_d = nc.dram_tensor("x", [SEQ, D_MODEL], f32, kind="ExternalInput")
    wqkv_d = nc.dram_tensor("wqkv", [D_MODEL, 576], f32, kind="ExternalInput")
    mqkv_d = nc.dram_tensor("mqkv", [D_MODEL, 576], f32, kind="ExternalInput")
    bqkv_d = nc.dram_tensor("bqkv", [5, 128], f32, kind="ExternalInput")
    wo_d = nc.dram_tensor("wo", [N_HEADS * D_HEAD, D_MODEL], f32, kind="ExternalInput")
    mo_d = nc.dram_tensor("mo", [N_HEADS * D_HEAD, D_MODEL], f32, kind="ExternalInput")
    bo_d = nc.dram_tensor("bo", [1, D_MODEL], f32, kind="ExternalInput")
    out_d = nc.dram_tensor("out", [CHQ, D_MODEL], f32, kind="ExternalOutput")

    with tile.TileContext(nc) as tc:
        with tc.tile_pool(name="const", bufs=1) as constp, \
             tc.tile_pool(name="dram", bufs=1, space="DRAM") as dramp:

            # ---- constants ----
            ident32 = constp.tile([128, 128], f32, tag="id32")
            make_identity(nc, ident32[:])
            ident_r = constp.tile([128, 128], bf16, tag="idr")
            nc.vector.tensor_copy(ident_r[:], ident32[:])
            # tri[p, f] = 1.0 if f >= p else 0.0  (inclusive-diagonal upper tri)
            tri = constp.tile([KT, KT], f32, tag="tri")
            nc.gpsimd.memset(tri[:], 1.0)
            nc.gpsimd.affine_select(
                out=tri[:], in_=tri[:], compare_op=mybir.AluOpType.is_ge,
                fill=0.0, base=0, channel_multiplier=-1, pattern=[[1, KT]])
            ones3 = constp.tile([128, HPC], f32, tag="ones3")
            nc.vector.memset(ones3[:], 1.0)
            ones1 = constp.tile([1, 128], f32, tag="ones1")
            nc.vector.memset(ones1[:], 1.0)
            ones_r = constp.tile([1, 128], mybir.dt.bfloat16, tag="ones_r")
            nc.vector.tensor_copy(ones_r[:], ones1[:])
            bias_sb = constp.tile([128, 5], f32, tag="bias")
            warm1 = constp.tile([1, 128], f32, tag="warm1")
            nc.scalar.activation(warm1[:], ones1[:], FT.Exp, scale=0.1)

            with tc.tile_pool(name="qkvt", bufs=1) as qkvtp, \
                 tc.tile_pool(name="vnat", bufs=1) as vnatp:
                qkvT = [qkvtp.tile([128, SEQ], bf16, tag=f"qkvT{i}", name=f"qkvT{i}")
                        for i in range(5)]
                vnat = [vnatp.tile([128, 65 * HPC], bf16, tag=f"vn{t}", name=f"vn{t}")
                        for t in range(NT)]

                with tc.tile_pool(name="xr", bufs=6) as xrp, \
                     tc.tile_pool(name="xT", bufs=1) as xTp, \
                     tc.tile_pool(name="wld", bufs=2) as wldp, \
                     tc.tile_pool(name="wr", bufs=1) as wrp, \
                     tc.tile_pool(name="psA", bufs=4, space="PSUM") as psA, \
                     tc.tile_pool(name="psB", bufs=3, space="PSUM") as psB:

                    xT = [xTp.tile([128, SEQ], bf16, tag=f"xT{d}", name=f"xT{d}")
                          for d in range(ND)]

                    # ones columns of vnat (col 65h+64 is the row-sum column)
                    for t in range(NT):
                        vv = vnat[t][:].rearrange("p (g c) -> p g c", c=65)
                        nc.vector.tensor_copy(vv[:, :, 64], ones3[:])

                    # ---- phase A: load x (HWDGE, f32), transpose to xT (f32r) ----
                    for t in range(NT):
                        xr = xrp.tile([128, D_MODEL], f32, tag="xr")
                        nc.sync.dma_start(out=xr[:], in_=x_d[KT * t:KT * (t + 1), :])
                        for d in range(ND):
                            pt = psA.tile([128, 128], f32, tag="pt")
                            nc.tensor.transpose(pt[:], xr[:, 128 * d:128 * (d + 1)], ident32[:])
                            if (t * ND + d) % 2 == 0:
                                nc.vector.tensor_copy(xT[d][:, KT * t:KT * (t + 1)], pt[:])
                            else:
                                nc.scalar.copy(xT[d][:, KT * t:KT * (t + 1)], pt[:])

                    # ---- phase B: masked weights, qkv projections ----
                    for mt in range(5):
                        nc.gpsimd.dma_start(out=bias_sb[:, mt:mt + 1], in_=bqkv_d[mt:mt + 1, :])
                    wr = []
                    for d in range(ND):
                        w32 = wldp.tile([128, 576], f32, tag="w32")
                        m32 = wldp.tile([128, 576], f32, tag="m32")
                        nc.sync.dma_start(out=w32[:], in_=wqkv_d[128 * d:128 * (d + 1), :])
                        nc.sync.dma_start(out=m32[:], in_=mqkv_d[128 * d:128 * (d + 1), :])
                        wrt = wrp.tile([128, 576], bf16, tag=f"wr{d}", name=f"wr{d}")
                        nc.vector.tensor_mul(wrt[:], w32[:], m32[:])
                        wr.append(wrt)
                    for mt in range(5):
                        M = 128 if mt < 4 else 64
                        for qn in range(NQC):
                            ps = psB.tile([128, CHQ], f32, tag="pproj")
                            for d in range(ND):
                                nc.tensor.matmul(
                                    ps[0:M, :], wr[d][:, 128 * mt:128 * mt + M],
                                    xT[d][:, CHQ * qn:CHQ * (qn + 1)],
                                    start=(d == 0), stop=(d == ND - 1))
                            nc.vector.tensor_scalar_add(
                                qkvT[mt][0:M, CHQ * qn:CHQ * (qn + 1)],
                                ps[0:M, :], bias_sb[0:M, mt:mt + 1])

                    # ---- phase C: v back to natural [keys, 64] layout ----
                    for h in range(HPC):
                        s = SLOT[("v", h)]
                        base = 64 * (s % 2)
                        vsrc = qkvT[s // 2][base:base + 64, :]
                        idb = ident_r[base:base + 64, base:base + 64]
                        for t in range(NT):
                            pv = psA.tile([128, 64], bf16, tag="pt")
                            nc.tensor.transpose(pv[:], vsrc[:, KT * t:KT * (t + 1)], idb)
                            nc.vector.tensor_copy(vnat[t][:, 65 * h:65 * h + 64], pv[:])

                # ---- phase D: attention, 3 heads interleaved per q-chunk ----
                with tc.tile_pool(name="za", bufs=1) as zap:
                    bf16 = mybir.dt.bfloat16
                    zA = [zap.tile([64, SEQ], bf16, tag=f"zA{h}", name=f"zA{h}")
                          for h in range(HPC)]
                    a2a_in = dramp.tile([N_CORES, HPC * 64, CHQ], bf16,
                                        tag="a2a_in", name="a2a_in")
                    a2a_out = dramp.tile([N_CORES, HPC * 64, CHQ], bf16,
                                         tag="a2a_out", name="a2a_out")
                    ones65_32 = zap.tile([65, 128], f32, tag="ones65_32")
                    nc.vector.memset(ones65_32[:], 1.0)
                    ones65_r = zap.tile([65, 128], f32r, tag="ones65_r")
                    nc.vector.tensor_copy(ones65_r[:], ones65_32[:])
                    hacc = []
                    for h in range(HPC):
                        qs, ks = SLOT[("q", h)], SLOT[("k", h)]
                        hacc.append((qkvT[qs // 2], 64 * (qs % 2),
                                     qkvT[ks // 2], 64 * (ks % 2)))
                    with tc.tile_pool(name="pstage", bufs=5) as pstp, \
                         tc.tile_pool(name="rcp", bufs=1) as rcpp, \
                         tc.tile_pool(name="psD", bufs=2, space="PSUM") as psD, \
                         tc.tile_pool(name="psZ", bufs=1, space="PSUM") as psZ, \
                         tc.tile_pool(name="psBC", bufs=1, space="PSUM") as psBC:
                        for qc in range(NQC):
                            zps = [psZ.tile([65, CHQ], f32, tag=f"zps{h}", name=f"zps{h}")
                                   for h in range(HPC)]
                            nkt = NKQ * qc + NKQ

                            def colo(kt, _qc=qc):
                                return (kt - NKQ * _qc) * KT if kt >= NKQ * _qc else 0

                            for pr in range(nkt // 2):
                                k0, k1 = 2 * pr, 2 * pr + 1
                                lo0, lo1 = colo(k0), colo(k1)
                                # S_T: h-inner so h0 (rows 0-63) and h1 (rows
                                # 64-127) run concurrently in the PE array
                                pps = [psD.tile([128, 2 * CHQ], f32, tag="pp",
                                                name=f"pp{h}") for h in range(HPC)]
                                for j, (kt, lo) in enumerate([(k0, lo0), (k1, lo1)]):
                                    for h in range(HPC):
                                        qT, qb, kT_, kb = hacc[h]
                                        nc.tensor.matmul(
                                            pps[h][:, CHQ * j + lo:CHQ * (j + 1)],
                                            kT_[kb:kb + 64, KT * kt:KT * (kt + 1)],
                                            qT[qb:qb + 64, CHQ * qc + lo:CHQ * (qc + 1)],
                                            start=True, stop=True)
                                Ps = []
                                for h in range(HPC):
                                    P = pstp.tile([128, 2 * CHQ], bf16, tag="P")
                                    nc.scalar.activation(P[:, lo0:], pps[h][:, lo0:],
                                                         FT.Exp, scale=SCALE)
                                    Ps.append(P)
                                for h in range(HPC):
                                    P = Ps[h]
                                    for j, (kt, lo) in enumerate([(k0, lo0), (k1, lo1)]):
                                        if kt >= NKQ * qc:
                                            nc.vector.tensor_mul(
                                                P[:, CHQ * j + lo:CHQ * j + lo + KT],
                                                P[:, CHQ * j + lo:CHQ * j + lo + KT],
                                                tri[:])
                                        nc.tensor.matmul(
                                            zps[h][:, lo:], vnat[kt][:, 65 * h:65 * (h + 1)],
                                            P[:, CHQ * j + lo:CHQ * (j + 1)],
                                            start=(kt == 0), stop=(kt == nkt - 1))
                            # normalize z rows 0-63 by 1/sums (head-parallel)
                            s65s, rc65s, rcr65s, bcs = [], [], [], []
                            for h in range(HPC):
                                s65 = rcpp.tile([65, CHQ], f32, tag=f"s65_{h}", name="s65")
                                nc.vector.tensor_copy(s65[:], zps[h][:])
                                s65s.append(s65)
                            for h in range(HPC):
                                rc65 = rcpp.tile([65, CHQ], f32, tag=f"rc65_{h}", name="rc65")
                                sc65 = rcpp.tile([65, CHQ], f32, tag="sc65", name="sc65")
                                nc.vector.reciprocal_approx_accurate(
                                    out=rc65[:], in_=s65s[h][:], scratch=sc65[:])
                                rc65s.append(rc65)
                            for h in range(HPC):
                                rcr65 = rcpp.tile([65, CHQ], f32r, tag=f"rcr65_{h}",
                                                  name="rcr65")
                                nc.vector.tensor_copy(rcr65[64:65, :], rc65s[h][64:65, :])
                                rcr65s.append(rcr65)
                            for h in range(HPC):
                                bc = psBC.tile([128, CHQ], f32, tag="bc", name="bc")
                                nc.tensor.matmul(bc[:], ones65_r[64:65, 0:128],
                                                 rcr65s[h][64:65, :], start=True, stop=True)
                                bcs.append(bc)
                            for h in range(HPC):
                                nc.vector.tensor_mul(
                                    zA[h][0:64, CHQ * qc:CHQ * (qc + 1)],
                                    s65s[h][0:64, :], bcs[h][0:64, :])
                            for h in range(HPC):
                                for d in (qc, qc + GROUP):
                                    nc.sync.dma_start(
                                        out=a2a_in[d, 64 * h:64 * (h + 1), :],
                                        in_=zA[h][0:64, CHQ * qc:CHQ * (qc + 1)])
            # ---- one 8-core AllToAll: each core receives exactly its q-shard ----
            nc.gpsimd.collective_compute(
                "AllToAll", mybir.AluOpType.bypass,
                replica_groups=[list(range(N_CORES))],
                ins=[a2a_in.opt()], outs=[a2a_out.opt()])

            # ---- phase E: query-sharded output projection ----
            with tc.tile_pool(name="zg", bufs=1) as zgp, \
                 tc.tile_pool(name="wo", bufs=2) as wop, \
                 tc.tile_pool(name="wor", bufs=1) as worp, \
                 tc.tile_pool(name="os", bufs=2) as osp, \
                 tc.tile_pool(name="psE", bufs=2, space="PSUM") as psE:
                rank = nc.sync.partition_id()
                aflat = a2a_out[:].rearrange("a b c -> (a b) c")
                roff = (rank // GROUP) * (GROUP * HPC * 64)
                zg = []
                for r in range(ND):
                    zt = zgp.tile([128, CHQ], bf16, tag=f"zg{r}", name=f"zg{r}")
                    nc.sync.dma_start(
                        out=zt[:], in_=aflat[bass.ds(roff + 128 * r, 128), :])
                    zg.append(zt)
                wo_r = []
                for r in range(ND):
                    w32 = wop.tile([128, D_MODEL], f32, tag="wo32")
                    m32 = wop.tile([128, D_MODEL], f32, tag="mo32")
                    nc.sync.dma_start(out=w32[:], in_=wo_d[128 * r:128 * (r + 1), :])
                    nc.sync.dma_start(out=m32[:], in_=mo_d[128 * r:128 * (r + 1), :])
                    wrt = worp.tile([128, D_MODEL], bf16, tag=f"wor{r}", name=f"wor{r}")
                    nc.vector.tensor_mul(wrt[:], w32[:], m32[:])
                    wo_r.append(wrt)
                bo32 = worp.tile([1, D_MODEL], f32, tag="bo32")
                nc.sync.dma_start(out=bo32[:], in_=bo_d[:])
                bor = worp.tile([1, D_MODEL], bf16, tag="bor")
                nc.vector.tensor_copy(bor[:], bo32[:])
                DC = D_MODEL // 2
                for t in range(CHQ // KT):
                    for dc in range(2):
                        ps = psE.tile([128, DC], f32, tag="po")
                        nc.tensor.matmul(ps[:], ones_r[:], bor[:, DC * dc:DC * (dc + 1)],
                                         start=True, stop=False)
                        for r in range(ND):
                            nc.tensor.matmul(
                                ps[:], zg[r][:, KT * t:KT * (t + 1)],
                                wo_r[r][:, DC * dc:DC * (dc + 1)],
                                start=False, stop=(r == ND - 1))
                        osb = osp.tile([128, DC], f32, tag="osb")
                        nc.vector.tensor_copy(osb[:], ps[:])
                        nc.sync.dma_start(
                            out=out_d[KT * t:KT * (t + 1), DC * dc:DC * (dc + 1)],
                            in_=osb[:])
    nc.compile()
    return nc


def _get_nc():
    global _BUILT
    if _BUILT is None:
        _BUILT = _build()
    return _BUILT


def _make_in_maps(inputs):
    f = np.float32
    x = np.ascontiguousarray(np.asarray(inputs["normalized_resid_pre"], dtype=f))
    W = {"q": np.asarray(inputs["W_Q"], f), "k": np.asarray(inputs["W_K"], f),
         "v": np.asarray(inputs["W_V"], f)}
    Mm = {"q": np.asarray(inputs["mask_W_Q"], f), "k": np.asarray(inputs["mask_W_K"], f),
          "v": np.asarray(inputs["mask_W_V"], f)}
    B = {"q": np.asarray(inputs["b_Q"], f), "k": np.asarray(inputs["b_K"], f),
         "v": np.asarray(inputs["b_V"], f)}
    wo = np.ascontiguousarray(np.asarray(inputs["W_O"], f).reshape(N_HEADS * D_HEAD, D_MODEL))
    mo = np.ascontiguousarray(np.asarray(inputs["mask_W_O"], f).reshape(N_HEADS * D_HEAD, D_MODEL))
    bo = np.asarray(inputs["b_O"], f).reshape(1, D_MODEL)

    in_maps = []
    for c in range(N_CORES):
        b, g = divmod(c, GROUP)
        heads = [HPC * g + i for i in range(HPC)]
        wqkv = np.zeros((D_MODEL, 576), f)
        mqkv = np.zeros((D_MODEL, 576), f)
        bqkv = np.zeros((5, 128), f)
        for s, (mat, hh) in enumerate(SLOTS):
            gh = heads[hh]
            wqkv[:, 64 * s:64 * (s + 1)] = W[mat][gh]
            mqkv[:, 64 * s:64 * (s + 1)] = Mm[mat][gh]
            bqkv[s // 2, 64 * (s % 2):64 * (s % 2) + 64] = B[mat][gh]
        in_maps.append({
            "x": np.ascontiguousarray(x[b]),
            "wqkv": wqkv, "mqkv": mqkv, "bqkv": bqkv,
            "wo": wo, "mo": mo, "bo": bo,
        })
    return in_maps


def _run(inputs, trace=False):
    from concourse.bass_utils import run_bass_kernel_spmd
    nc = _get_nc()
    res = run_bass_kernel_spmd(nc, _make_in_maps(inputs),
                               core_ids=list(range(N_CORES)), trace=trace)
    out = np.empty((BATCH, SEQ, D_MODEL), np.float32)
    for c in range(N_CORES):
        b, g = divmod(c, GROUP)
        out[b, CHQ * g:CHQ * (g + 1), :] = res.results[c]["out"]
    return out, res


def kernel(**inputs):
    out, _ = _run(inputs, trace=False)
    return out
